# revision 5
# baseline (speedup 1.0000x reference)
"""VQ codebook nearest-neighbor kernel for TRN2 (8 NeuronCores, data-parallel).

argmin_k ||z - c_k||^2 == argmax_k (2 z.c_k - |c_k|^2), computed per core over
8192 tokens (z sharded along the flattened token dim across the 8 cores; the
1024x64 codebook replicated on every core).

Per group of 8 x 128-token tiles: one 2KB-contiguous z load, one fp16 hi/lo
split (ACT + alternating DVE/GPSIMD), then per tile a single fused PE
transpose ([zh|zl] -> [zh^T; zl^T] in one [128,128] fp16 instruction), 4
matmuls (hi stacked C=128 + lo C=64, per 512-col psum half), and a 1-pass
custom-DVE argmax straight from PSUM. Indices convert to int32 on ACT and
feed per-partition indirect row gathers (one offset per partition per
instruction -- multi-offset gathers corrupt on real HW).

Preamble: PE kept continuously busy from t=0 (p-state anchor), codebook
loaded first on SP, fp16 hi/lo of 2C built pre-transpose and transposed via
fp16 slabs (f32 transposes cannot write psum partition 64), and csq
broadcast down partitions with one-hot C=8 fp16 hi/lo matmuls.

Steady state is bound by the DVE argmax pass (~1.19us per 128x1024 tile;
custom DVE ops get no 2x/4x perf modes) and the per-tile GPSIMD gather prep.
"""

import sys

sys.path.insert(0, "/opt/trn_rl_repo")

import numpy as np

import concourse.bass as bass
import concourse.bacc as bacc
import concourse.mybir as mybir
from concourse.tile import TileContext
from concourse.bass_utils import run_bass_kernel_spmd
from concourse.masks import make_identity

import concourse.dve_ops as dve_ops
from concourse.dve_ops import DveOp
from concourse.dve_spec import (
    Spec,
    Src0,
    Src1,
    MaxNeg,
    Idx,
    Bin,
    AluOp,
    select,
    maxx,
    lower,
    Scan,
    _has_src1,
)
from concourse.dve_uop import DveOpSpec

# ---------------------------------------------------------------------------
# problem constants (hardcoded per harness contract)
B, T, D = 32, 2048, 64
K = 1024
N_CORES = 8
NTOK = B * T
TOK_PER_CORE = NTOK // N_CORES  # 8192
TILES = TOK_PER_CORE // 128  # 64
GROUP = 8  # tiles per load/gather/store batch
N_GROUPS = TILES // GROUP

F32 = mybir.dt.float32
FP16 = mybir.dt.float16
I32 = mybir.dt.int32

FLT_MAX_NEG = np.float32(-3.4028235e38)


# ---------------------------------------------------------------------------
# custom fused-argmax DVE op: accum_out[p] = argmax_f (in0[p,f] - in1[p,f])
def _argmax_sub_ref(in0, in1, s0, s1, imm2):
    x = (in0.astype(np.float32) - in1.astype(np.float32)).reshape(in0.shape[0], -1)
    run = np.maximum.accumulate(x, axis=1)
    idx = np.arange(x.shape[1], dtype=np.float32)[None, :]
    body = np.where(x >= run, idx, FLT_MAX_NEG).astype(np.float32)
    acc = body.max(axis=1, keepdims=True)
    return body.reshape(in0.shape), acc


def _make_argmax_op():
    for op in dve_ops.OPS:
        if op.name == "ARGMAX_SUB_ANT":
            return op
    x = Bin(AluOp.SUBTRACT, Src0, Src1)
    run = Scan(AluOp.MAX, x)
    body = select(Bin(AluOp.IS_GE, x, run), Idx, MaxNeg)
    spec = Spec(body=body, accum=maxx, accum_init=MaxNeg, reference=_argmax_sub_ref)
    opcode = dve_ops._CUSTOM_DVE_ROW_BASE + len(dve_ops.OPS)
    shas = {}
    for ver in ("v3", "v4"):
        uops = lower(spec, ver=ver)
        s = DveOpSpec(name="ARGMAX_SUB_ANT", opcode=opcode, uops=uops,
                      rd1_en=_has_src1(spec))
        shas[ver] = s.sha(ver)
    op = DveOp("ARGMAX_SUB_ANT", spec, subdim=False, uops_sha=shas)
    dve_ops.OPS.append(op)
    dve_ops.CUSTOM_DVE_SPECS[op.name] = op.spec
    dve_ops._SUB_OPCODE_FOR_NAME[op.name] = opcode
    return op


ARGMAX_SUB = _make_argmax_op()


# ---------------------------------------------------------------------------
def _build_kernel():
    nc = bacc.Bacc(trn_type="TRN2", target_bir_lowering=False, debug=False)
    z = nc.dram_tensor("z", [TOK_PER_CORE, D], F32, kind="ExternalInput")
    cb = nc.dram_tensor("codebook", [K, D], F32, kind="ExternalInput")
    out = nc.dram_tensor("out", [TOK_PER_CORE, D], F32, kind="ExternalOutput")

    with TileContext(nc) as tc:
        with (
            tc.tile_pool(name="const", bufs=1) as cpool,
            tc.tile_pool(name="zin", bufs=2) as zpool,
            tc.tile_pool(name="work", bufs=3) as pool,
            tc.tile_pool(name="gath", bufs=2) as gpool,
            tc.tile_pool(name="scratch", bufs=3) as spool,
            tc.tile_pool(name="psum_s", bufs=3, space="PSUM") as psum_s,
            tc.tile_pool(name="psum_t", bufs=2, space="PSUM") as psum_t,
        ):
            ident = cpool.tile([128, 128], F32)
            make_identity(nc, ident[:])
            ident16 = cpool.tile([128, 128], FP16, tag="ident16")
            make_identity(nc, ident16[:])

            # ---- z group load (DMA only; split is issued separately) ----
            def issue_load(g):
                tok0 = g * GROUP * 128
                zsb = zpool.tile([128, GROUP * D], F32, tag="zsb")
                # token (g*GROUP*128 + p*GROUP + j) -> zsb[p, j, :]
                nc.sync.dma_start(
                    zsb[:].rearrange("p (j d) -> p j d", j=GROUP),
                    z[tok0:tok0 + GROUP * 128, :].rearrange(
                        "(p j) d -> p j d", p=128
                    ),
                )
                return zsb

            g_split = [0]

            def issue_split(zsb):
                # fp16 hi/lo split, laid out [p, j, {hi,lo}, d] so each
                # tile's [zh_j | zl_j] block is CONTIGUOUS (the fused
                # transpose needs a single-free-dim rhs AP on real HW)
                zhl = zpool.tile([128, 2 * GROUP * D], FP16, tag="zhl")
                zhl4 = zhl[:].rearrange("p (j k d) -> p j k d", j=GROUP, k=2)
                nc.scalar.copy(zhl4[:, :, 0, :], zsb[:].rearrange(
                    "p (j d) -> p j d", j=GROUP))
                eng = nc.vector if (g_split[0] % 2 == 0) else nc.gpsimd
                g_split[0] += 1
                eng.tensor_sub(
                    zhl4[:, :, 1, :],
                    zsb[:].rearrange("p (j d) -> p j d", j=GROUP),
                    zhl4[:, :, 0, :])
                return zhl

            # E8[r, kc*128+j] = (r == kc): one-hot rows for the csq broadcast
            # matmuls (lhsT slices stay at base partition 0, unlike row
            # slicing of the csq matrix which the matmul API rejects).
            E8 = cpool.tile([8, K], FP16, tag="E8")
            nc.gpsimd.memset(E8[:], 0.0)
            # E8[r, c] = (c // 128 == r), built with two affine selects:
            # fill 1.0 where c - 128r >= 0, then zero where c - 128r > 127
            nc.gpsimd.affine_select(
                out=E8[:], in_=E8[:], compare_op=mybir.AluOpType.is_gt,
                fill=1.0, base=0, pattern=[[-1, K]], channel_multiplier=128,
            )
            nc.gpsimd.affine_select(
                out=E8[:], in_=E8[:], compare_op=mybir.AluOpType.is_ge,
                fill=0.0, base=127, pattern=[[-1, K]], channel_multiplier=128,
            )
            wsrc = cpool.tile([128, 512], FP16, tag="wsrc")
            nc.gpsimd.memset(wsrc[:], 0.0)

            # PE p-state warmup: pe_busy_start re-anchors whenever PE goes
            # idle->busy, so PE must be kept CONTINUOUSLY busy from ~t=0
            # until the codebook lands (~4.3us) for the preamble transposes
            # and matmuls to be costed at the fully-ramped clock.
            for _ in range(6):
                wp = psum_t.tile([128, 512], F32, tag="zT")
                nc.tensor.matmul(wp[:], ident16[:], wsrc[:],
                                 start=True, stop=True)

            # cb load FIRST on the SP queue (it gates the whole preamble),
            # then the group-0 z load.
            cbt_all = cpool.tile([128, 8 * D], F32, tag="cb_load")
            nc.sync.dma_start(
                cbt_all[:].rearrange("p (kc d) -> p kc d", kc=8),
                cb[:, :].rearrange("(kc p) d -> p kc d", p=128),
            )
            zsb_cur = issue_load(0)

            # hi/lo fp16 split of 2C BEFORE any transpose (the PE transpose
            # moves raw data -- identity values are NOT multiplied on real
            # HW, so the x2 must be materialized): c2 = 2C (f32, exact),
            # ch1 = fp16(2C), l2 = fp16(2C - ch1).
            c2_all = cpool.tile([128, 8 * D], F32, tag="c2_all")
            nc.scalar.mul(c2_all[:], cbt_all[:], 2.0)
            ch1 = cpool.tile([128, 8 * D], FP16, tag="ch1")
            nc.scalar.copy(ch1[:], c2_all[:])
            zhl_cur = issue_split(zsb_cur)

            sq_all = cpool.tile([128, 8 * D], F32, tag="sq_all")
            nc.vector.tensor_mul(sq_all[:], cbt_all[:], cbt_all[:])
            csq_pk = cpool.tile([128, 8], F32, tag="csq_pk")
            nc.vector.tensor_reduce(
                csq_pk[:],
                sq_all[:].rearrange("p (kc d) -> p kc d", kc=8),
                axis=mybir.AxisListType.X,
                op=mybir.AluOpType.add,
            )
            l2 = cpool.tile([128, 8 * D], FP16, tag="l2")
            nc.vector.tensor_sub(l2[:], c2_all[:], ch1[:])

            # chT2 = [(2C)h^T; (2C)h^T] stacked: 8 fp16 transposes per
            # [128, 512] psum slab (partitions 0/64, free offsets 0..384 --
            # all legal for fp16 transposes), one copy per slab (DVE / ACT).
            chT2 = cpool.tile([128, K], FP16, tag="chT2")
            for hs in range(2):
                slab = psum_t.tile([128, 512], FP16, tag="zT")
                for k4 in range(4):
                    kc = hs * 4 + k4
                    for half in range(2):
                        nc.tensor.transpose(
                            slab[half * 64:(half + 1) * 64,
                                 k4 * 128:(k4 + 1) * 128],
                            ch1[:, kc * D:(kc + 1) * D], ident16[:]
                        )
                dst = chT2[:, hs * 512:(hs + 1) * 512]
                if hs == 0:
                    nc.vector.tensor_scalar_add(dst, slab[:], 0.0)
                else:
                    nc.scalar.copy(dst, slab[:])

            # csq: one f32 transpose [128,8] -> [8,128] at partition 0
            pqT = psum_t.tile([8, 128], F32, tag="zT")
            nc.tensor.transpose(pqT[:], csq_pk[:], ident[:])
            csqh = cpool.tile([8, 128], FP16, tag="csqh")
            nc.vector.tensor_scalar_mul(csqh[:], pqT[:], 1.0)
            csql = cpool.tile([8, 128], FP16, tag="csql")
            csqm = cpool.tile([8, 128], F32, tag="csqm")
            nc.vector.tensor_scalar_mul(csqm[:], pqT[:], 1.0)
            nc.vector.tensor_sub(csql[:], csqm[:], csqh[:])

            # clT = l2^T (top 64 partitions only)
            clT = cpool.tile([64, K], FP16, tag="clT")
            for hs in range(2):
                slab = psum_t.tile([64, 512], FP16, tag="zT")
                for k4 in range(4):
                    kc = hs * 4 + k4
                    nc.tensor.transpose(
                        slab[:, k4 * 128:(k4 + 1) * 128],
                        l2[:, kc * D:(kc + 1) * D], ident16[:]
                    )
                dst = clT[:, hs * 512:(hs + 1) * 512]
                if hs == 0:
                    nc.vector.tensor_scalar_add(dst, slab[:], 0.0)
                else:
                    nc.scalar.copy(dst, slab[:])

            # csq broadcast: per kc two accumulating C=8 fp16 matmuls with
            # one-hot lhsT slices replicate csq row kc (split into fp16
            # hi+lo, exact to ~2^-22) down all 128 partitions.
            csq_rep = cpool.tile([128, K], F32, tag="csq_rep")
            pcs = psum_s.tile([128, K], F32, tag="scores")
            for kc in range(8):
                ksl = slice(kc * 128, (kc + 1) * 128)
                nc.tensor.matmul(pcs[:, ksl], E8[:, ksl], csqh[:],
                                 start=True, stop=False)
                nc.tensor.matmul(pcs[:, ksl], E8[:, ksl], csql[:],
                                 start=False, stop=True)
            # csq_rep psum->sbuf off the ACT queue (it would delay the first
            # zaT copies): half on DVE, half on the Pool engine
            nc.vector.tensor_scalar_add(csq_rep[:, 0:512], pcs[:, 0:512], 0.0)
            nc.scalar.copy(csq_rep[:, 512:1024], pcs[:, 512:1024])

            # ---- main loop over groups of GROUP tiles ----
            zsb_next = None
            for g in range(N_GROUPS):
                zhl4 = zhl_cur[:].rearrange(
                    "p (j k d) -> p j k d", j=GROUP, k=2
                )
                tok0 = g * GROUP * 128
                if g + 1 < N_GROUPS:
                    zsb_next = issue_load(g + 1)

                idxf = gpool.tile([128, GROUP], F32, tag="idxf")
                idxi = gpool.tile([128, GROUP], I32, tag="idxi")
                gout = gpool.tile([128, GROUP * D], F32, tag="gout")
                zhl_next = None
                for j in range(GROUP):
                    # one fused transpose: [zh_j | zl_j] -> [zh_j^T; zl_j^T]
                    pzT = psum_t.tile([128, 128], FP16, tag="zT")
                    nc.tensor.transpose(pzT[:], zhl4[:, j, :, :], ident16[:])
                    zaT = pool.tile([128, 128], FP16, tag="zaT")
                    nc.scalar.copy(zaT[:], pzT[:])

                    ps = psum_s.tile([128, K], F32, tag="scores")
                    for h in range(2):
                        hs = slice(h * 512, (h + 1) * 512)
                        nc.tensor.matmul(ps[:, hs], zaT[:, :], chT2[:, hs],
                                         start=True, stop=False)
                        nc.tensor.matmul(ps[:, hs], zaT[0:64, :], clT[:, hs],
                                         start=False, stop=True)
                    scratch = spool.tile([128, K], F32, tag="amx_scratch")
                    nc.vector._custom_dve(
                        ARGMAX_SUB,
                        out=scratch[:],
                        in0=ps[:],
                        in1=csq_rep[:],
                        accum_out=idxf[:, j:j + 1],
                    )
                    nc.scalar.copy(idxi[:, j:j + 1], idxf[:, j:j + 1])
                    if j == 4 and g + 1 < N_GROUPS:
                        # split next group mid-group: its DMA has landed, and
                        # Pool ordering stays [split(g+1), gather(g)]
                        zhl_next = issue_split(zsb_next)
                    if True:
                        # last group: per-tile gather+store, pipelined with the
                        # remaining argmaxes, so the post-loop tail is only one
                        # 128-row gather+store chain instead of a full group's
                        nc.gpsimd.indirect_dma_start(
                            out=gout[:].rearrange(
                                "p (j d) -> p j d", j=GROUP
                            )[:, j, :],
                            out_offset=None,
                            in_=cb[:, :],
                            in_offset=bass.IndirectOffsetOnAxis(
                                ap=idxi[:, j:j + 1], axis=0
                            ),
                        )
                        if g == N_GROUPS - 1:
                            # last group: per-tile stores keep the post-loop
                            # tail to one 128-row store chain
                            nc.sync.dma_start(
                                out[tok0:tok0 + GROUP * 128, :].rearrange(
                                    "(p j) d -> p j d", p=128
                                )[:, j, :],
                                gout[:].rearrange(
                                    "p (j d) -> p j d", j=GROUP
                                )[:, j, :],
                            )
                if g < N_GROUPS - 1:
                    # one contiguous 2KB-per-partition store per group
                    nc.sync.dma_start(
                        out[tok0:tok0 + GROUP * 128, :].rearrange(
                            "(p j) d -> p j d", p=128
                        ),
                        gout[:].rearrange("p (j d) -> p j d", j=GROUP),
                    )
                zhl_cur = zhl_next

    nc.compile()
    return nc


_NC_CACHE = None


def _get_nc():
    global _NC_CACHE
    if _NC_CACHE is None:
        _NC_CACHE = _build_kernel()
    return _NC_CACHE


def kernel(z: np.ndarray, codebook: np.ndarray) -> np.ndarray:
    nc = _get_nc()
    z = np.ascontiguousarray(z, dtype=np.float32)
    codebook = np.ascontiguousarray(codebook, dtype=np.float32)
    z_flat = z.reshape(-1, D)
    shards = np.split(z_flat, N_CORES, axis=0)
    in_maps = [{"z": s, "codebook": codebook} for s in shards]
    res = run_bass_kernel_spmd(nc, in_maps, core_ids=list(range(N_CORES)))
    out = np.concatenate([res.results[c]["out"] for c in range(N_CORES)], axis=0)
    return out.reshape(z.shape)
